# revision 26
# baseline (speedup 1.0000x reference)
"""MemristorDense Trainium2 kernel (8 NeuronCores, SPMD tensor-parallel).

Math: y[b,o] = (I[b,2o] - I[b,2o+1]) / (K_V*k_G), where
  I[b,j]  = sum_i Gv[i,j] * ratio[b,i]^E[i,j],
  Gv      = (k_G*|combined| + G_MIN)*V_REF,  E = log2(n_devices),
  ratio   = 2*inputs (inputs = [x, 1]), k_G = (G_MAX-G_MIN)/max|combined|.

k_G cancels in y:  y = 0.5 * diff_j( sum_i (|w[i,j]| + mw/99) * ratio^E ),
with mw = max|combined|.  The powers are evaluated with a separable series
  ratio^E = e^{L*E} = e^{L*mu} * sum_k (L)^k * (d^k/k!),   L = ln(ratio),
  d = E - mu,
which turns the [B,1025,1024] elementwise-pow + reduction into K+1 small
matmuls per core:  I = sum_k C_k^T @ W_k with
  C_k[i,b] = e^{mu*L[b,i]} * L[b,i]^k      (recurrence C_k = C_{k-1}*L)
  W_k[i,j] = W_0[i,j] * d[i,j]^k / k!      (recurrence W_k = W_{k-1}*D_k,
                                            D_k = D_{k-1}*(k-1)/k, D_1 = d)
Sharding: each core owns 128 of the 1024 interleaved columns (=64 outputs).
The global max mw is computed redundantly per-core from a bf16 replica of
combined (mw only feeds the ~1% G_MIN correction, bf16 rounding of the max
is far below fp32 noise).
"""

from contextlib import ExitStack

import ml_dtypes
import numpy as np

import concourse.bass as bass
import concourse.bass_isa as bass_isa
import concourse.tile as tile
from concourse import bacc
from concourse import mybir
from concourse import bass_utils

P = 128
B = 128
N_IN = 1024
N_OUT = 512
NJ = 2 * N_OUT          # 1024 interleaved columns
NCH = 9                 # i-chunks of 128 (1025 rows padded to 1152)
IPAD = NCH * P          # 1152
JC = NJ // 8            # 128 columns per core
NCORES = 8
K_TERMS = 6             # series terms k = 0..6

MU = 1.58               # expansion center for E = log2(n)
LN2 = float(np.log(2.0))
C_GMIN = 1.0 / 99.0     # G_MIN/(G_MAX - G_MIN)
L_CLAMP = -200.0        # ln(ratio) clamp; e^{mu*L_CLAMP} underflows to 0

F32 = mybir.dt.float32
BF16 = mybir.dt.bfloat16
AF = mybir.ActivationFunctionType
ALU = mybir.AluOpType

_NC_CACHE = None


def _kernel_body(ctx, tc, combm, slice32, xt, nsl, y):
    nc = tc.nc
    FR = IPAD  # 1152 free elems for [i-chunk-blocked] tiles

    const = ctx.enter_context(tc.tile_pool(name="const", bufs=1))
    big = ctx.enter_context(tc.tile_pool(name="big", bufs=1))
    wpool = ctx.enter_context(tc.tile_pool(name="wpool", bufs=7))
    cpool = ctx.enter_context(tc.tile_pool(name="cpool", bufs=7))
    dpool = ctx.enter_context(tc.tile_pool(name="dpool", bufs=4))
    psum = ctx.enter_context(tc.tile_pool(name="psum", bufs=1, space="PSUM"))

    # ---- loads; DRAM is host-pre-blocked [p, c, f] so each partition's
    # data is one contiguous run (128 large descriptors per DMA).
    # Two HWDGE queues: cb chunks on qSP (nc.sync), rest on qAct
    # (nc.scalar), xt first so the C-chain starts early. ----
    sl = big.tile([P, NCH, JC], F32, tag="sl")
    nc.scalar.dma_start(sl[:], slice32.ap())
    xtile = big.tile([P, NCH - 1, B], F32, tag="xt")
    nc.scalar.dma_start(xtile[:], xt.ap())
    ns = big.tile([P, NCH, JC], F32, tag="ns")
    nc.scalar.dma_start(ns[:], nsl.ap())
    cb = big.tile([P, NCH, NJ], BF16, tag="cb")
    for c in range(NCH):
        nc.sync.dma_start(cb[:, c, :], combm.ap()[:, c, :])

    # ---- global max |combined| (bf16 replica) -> per-partition bias c*mw --
    rm = const.tile([P, 1], F32, tag="rm")
    nc.vector.tensor_reduce(
        rm[:], cb[:], axis=mybir.AxisListType.XY, op=ALU.max,
        apply_absolute_value=True,
    )
    mwall = const.tile([P, 1], F32, tag="mwall")
    nc.gpsimd.partition_all_reduce(
        mwall[:], rm[:], channels=P, reduce_op=bass_isa.ReduceOp.max
    )
    cmw = const.tile([P, 1], F32, tag="cmw")
    # fold the final output scale 0.5 here: bias = 0.5 * (mw/99)
    nc.scalar.mul(cmw[:], mwall[:], 0.5 * C_GMIN)

    # ---- L = ln(2*x), blocked [i, b]; bias row L=ln2; pad rows clamped ----
    lt = const.tile([P, NCH, B], F32, tag="lt")
    nc.scalar.activation(lt[:, 0:NCH - 1, :], xtile[:], AF.Ln, bias=0.0, scale=2.0)
    nc.vector.tensor_scalar_max(lt[:, 0:NCH - 1, :], lt[:, 0:NCH - 1, :], L_CLAMP)
    nc.any.memset(lt[:, NCH - 1, :], L_CLAMP)    # i > 1024: zero-pad rows
    nc.any.memset(lt[0:1, NCH - 1, :], LN2)      # i = 1024: bias input row

    # ---- C_0 = e^{mu*L} ----
    c0 = cpool.tile([P, NCH, B], F32, tag="ck")
    nc.scalar.activation(c0[:], lt[:], AF.Exp, bias=0.0, scale=MU)

    # ---- d = log2(n) - mu = ln(n)/ln2 - mu ----
    lnn = dpool.tile([P, NCH, JC], F32, tag="lnn")
    nc.scalar.activation(lnn[:], ns[:], AF.Ln, bias=0.0, scale=1.0)
    d1 = dpool.tile([P, NCH, JC], F32, tag="dk")
    nc.vector.tensor_scalar(d1[:], lnn[:], 1.0 / LN2, -MU, op0=ALU.mult, op1=ALU.add)

    # ---- W_0 = 0.5*|w| + 0.5*mw/99 (0.5 = final V_REF/K_V scale) ----
    ab = wpool.tile([P, NCH, JC], F32, tag="ab")
    nc.scalar.activation(ab[:], sl[:], AF.Abs, bias=0.0, scale=0.5)
    w0 = wpool.tile([P, NCH, JC], F32, tag="wk")
    nc.scalar.activation(w0[:], ab[:], AF.Identity, bias=cmw[:], scale=1.0)

    # ---- series: PSUM += C_k^T @ W_k over k and i-chunks ----
    ps = psum.tile([P, JC], F32, tag="acc")
    wk, ck, dk = w0, c0, d1
    for k in range(K_TERMS + 1):
        if k > 0:
            if k > 1:
                dn = dpool.tile([P, NCH, JC], F32, tag="dk")
                nc.scalar.mul(dn[:], dk[:], float(k - 1) / float(k))
                dk = dn
            wn = wpool.tile([P, NCH, JC], F32, tag="wk")
            nc.vector.tensor_mul(wn[:], wk[:], dk[:])
            cn = cpool.tile([P, NCH, B], F32, tag="ck")
            nc.gpsimd.tensor_mul(cn[:], ck[:], lt[:])
            wk, ck = wn, cn
        for c in range(NCH):
            nc.tensor.matmul(
                ps[:],
                lhsT=ck[:, c, :],
                rhs=wk[:, c, :],
                start=(k == 0 and c == 0),
                stop=(k == K_TERMS and c == NCH - 1),
            )

    # ---- y = even - odd columns (x0.5 already folded into W_0) ----
    sb = const.tile([P, JC], F32, tag="sb")
    nc.scalar.copy(sb[:], ps[:])
    yt = const.tile([P, JC // 2], F32, tag="yt")
    sb3 = sb[:].rearrange("p (j two) -> p j two", two=2)
    nc.vector.tensor_sub(yt[:], sb3[:, :, 0], sb3[:, :, 1])
    nc.scalar.dma_start(y.ap(), yt[:])


def build_nc(repeat=1):
    nc = bacc.Bacc(
        "TRN2", target_bir_lowering=False, debug=False, num_devices=NCORES
    )
    combm = nc.dram_tensor("combm", [P, NCH, NJ], BF16, kind="ExternalInput")
    slice32 = nc.dram_tensor("slice32", [P, NCH, JC], F32, kind="ExternalInput")
    xt = nc.dram_tensor("xt", [P, NCH - 1, B], F32, kind="ExternalInput")
    nsl = nc.dram_tensor("nsl", [P, NCH, JC], F32, kind="ExternalInput")
    y = nc.dram_tensor("y", [B, JC // 2], F32, kind="ExternalOutput")
    with tile.TileContext(nc) as tc:
        with ExitStack() as ctx:
            if repeat == 1:
                _kernel_body(ctx, tc, combm, slice32, xt, nsl, y)
            else:
                with tc.For_i(0, repeat, 1):
                    _kernel_body(ctx, tc, combm, slice32, xt, nsl, y)
    nc.compile()
    return nc


def _block(a):
    """[NCH*P, W] row-major -> [P, NCH, W] partition-major contiguous."""
    n, w = a.shape
    ch = n // P
    return np.ascontiguousarray(a.reshape(ch, P, w).transpose(1, 0, 2))


def make_in_maps(x, w_pos, w_neg, b_pos, b_neg, n_devices):
    comb = np.zeros((IPAD, NJ), np.float32)
    comb[:N_IN, 0::2] = w_pos
    comb[:N_IN, 1::2] = w_neg
    comb[N_IN, 0::2] = b_pos
    comb[N_IN, 1::2] = b_neg
    combm = _block(comb.astype(ml_dtypes.bfloat16))
    xtA = _block(np.ascontiguousarray(np.asarray(x, np.float32).T))
    nsl = np.full((IPAD, NJ), 2.0, np.float32)
    nsl[:N_IN + 1] = n_devices
    in_maps = []
    for c in range(NCORES):
        js = slice(JC * c, JC * (c + 1))
        in_maps.append({
            "combm": combm,
            "slice32": _block(np.ascontiguousarray(comb[:, js])),
            "xt": xtA,
            "nsl": _block(np.ascontiguousarray(nsl[:, js])),
        })
    return in_maps


def gather(results):
    return np.concatenate(
        [np.asarray(results[c]["y"], np.float32) for c in range(NCORES)], axis=1
    )


def _get_nc():
    global _NC_CACHE
    if _NC_CACHE is None:
        _NC_CACHE = build_nc()
    return _NC_CACHE


def kernel(x, w_pos, w_neg, b_pos, b_neg, n_devices):
    in_maps = make_in_maps(x, w_pos, w_neg, b_pos, b_neg, n_devices)
    res = bass_utils.run_bass_kernel_spmd(
        _get_nc(), in_maps, core_ids=list(range(NCORES))
    )
    return gather(res.results)
